# revision 1
# baseline (speedup 1.0000x reference)
"""AttentiveFP forward on 8 Trainium2 NeuronCores (Bass/Tile).

Sharding: the 2048 graphs (and their nodes, contiguous since batch is sorted)
are split into 8 blocks of 256 graphs; each core owns the edges whose dst node
falls in its block. Per message-passing round each core computes its nodes'
features, all-gathers the per-node table [xt | alpha_src] across the 8 cores,
then runs gather / segment-softmax / weighted-segment-sum for its local edges:

- per-edge source rows are fetched 128-rows-per-call with indirect DMA
  (slots dst-sorted, 128-slot chunks aligned to 128-node tiles)
- segment softmax + weighted segment sums are PSUM-accumulated one-hot
  matmuls; the one-hot is built on DVE from host-precomputed relative-dst ids
- node-level GRU / linear phases run feature-major ("transposed"), nodes
  half-packed across the 128 partitions (features 0-63 on partitions 0-63 for
  the first half of nodes, on partitions 64-127 for the second half)

Softmax max-subtraction is skipped: softmax is shift-invariant and the
logits here are O(1), so exp() cannot overflow and the 1e-16 denominator
epsilon is negligible relative to sums of exps.
"""
import sys
sys.path.insert(0, '/opt/trn_rl_repo')
sys.path.insert(0, '/root/.axon_site')

import numpy as np

NC = 8
D = 64
G_TOT = 2048
G_LOC = G_TOT // NC
F_IN = 25
E_DIM = 4
NEG = 0.01
P = 128
W1 = D + 1     # gather-table row width


def _prep(x, edge_index, edge_attr, batch):
    N = x.shape[0]
    src = edge_index[0].astype(np.int64)
    dst = edge_index[1].astype(np.int64)
    batch = batch.astype(np.int64)

    gstart = np.searchsorted(batch, np.arange(0, G_TOT + 1, G_LOC))
    n0 = gstart[:-1]
    nloc = np.diff(gstart)
    n_pad = int(np.ceil((nloc.max() + 1) / 256) * 256)   # multiple of 256
    t_tiles = n_pad // P

    src_dev = np.searchsorted(gstart[1:], src, side='right')
    dst_dev = np.searchsorted(gstart[1:], dst, side='right')
    gidx_all = src_dev * n_pad + (src - n0[src_dev])

    per = []
    for c in range(NC):
        sel = np.where(dst_dev == c)[0]
        dl = dst[sel] - n0[c]
        order = np.argsort(dl, kind='stable')
        sel = sel[order]
        dl = dl[order]
        gi = gidx_all[sel]
        at = edge_attr[sel].astype(np.float32)
        tile_of = dl // P
        idx_cols, rel_cols, attr_cols, tile_chunks = [], [], [], []
        for t in range(t_tiles):
            m = tile_of == t
            k = int(m.sum())
            nch = max(1, (k + P - 1) // P)
            pad = nch * P - k
            gi_t = np.concatenate([gi[m], np.zeros(pad, np.int64)])
            rel_t = np.concatenate([(dl[m] - t * P).astype(np.float32),
                                    np.full(pad, 255.0, np.float32)])
            at_t = np.concatenate([at[m], np.zeros((pad, E_DIM), np.float32)], 0)
            idx_cols.append(gi_t.reshape(nch, P).T)
            rel_cols.append(rel_t.reshape(nch, P).T)
            attr_cols.append(at_t.reshape(nch, P, E_DIM).transpose(1, 0, 2)
                             .reshape(P, nch * E_DIM))
            tile_chunks.append(nch)
        per.append(dict(idx_cols=idx_cols, rel_cols=rel_cols, attr_cols=attr_cols,
                        tile_chunks=tile_chunks))
        # graph-relative ids per node vs psum graph-tile bases 0 / 128
        gl = batch[n0[c]:n0[c] + nloc[c]] - G_LOC * c
        gl = np.concatenate([gl, np.full(n_pad - nloc[c], 100000, np.int64)])
        per[c]['grel0'] = gl.astype(np.float32).reshape(t_tiles, P).T.copy()
        per[c]['grel1'] = (gl - P).astype(np.float32).reshape(t_tiles, P).T.copy()
        # half-packed feature-major input x
        xp = np.zeros((n_pad, F_IN), np.float32)
        xp[:nloc[c]] = x[n0[c]:n0[c] + nloc[c]]
        h = n_pad // 2
        xfm = np.zeros((32 + F_IN, h), np.float32)
        xfm[:F_IN] = xp[:h].T
        xfm[32:32 + F_IN] = xp[h:].T
        per[c]['xfm'] = xfm
    # identical per-tile chunk counts across cores (same SPMD program)
    tc_max = [max(per[c]['tile_chunks'][t] for c in range(NC)) for t in range(t_tiles)]
    CH = sum(tc_max)
    for c in range(NC):
        idx_n = np.zeros((P, CH), np.int32)
        rel_n = np.full((P, CH), 255.0, np.float32)
        attr_n = np.zeros((P, CH * E_DIM), np.float32)
        s = 0
        for t in range(t_tiles):
            k = per[c]['tile_chunks'][t]
            idx_n[:, s:s + k] = per[c]['idx_cols'][t]
            rel_n[:, s:s + k] = per[c]['rel_cols'][t]
            attr_n[:, s * E_DIM:(s + k) * E_DIM] = per[c]['attr_cols'][t]
            s += tc_max[t]
        per[c]['idx'], per[c]['rel'], per[c]['attr'] = idx_n, rel_n, attr_n
    return per, n_pad, t_tiles, tc_max, CH


def _mk_weights(kw):
    """Device-layout weights: feature-contraction lhsT tiles stacked twice
    (rows [0:K] == rows [K:2K]) so both node-halves slice at their base."""
    w = {}
    def stack(a):                       # [K, M] -> [2K, M]
        return np.concatenate([a, a], 0)
    def col(a):                         # [M] -> [2M, 1] stacked bias
        return np.concatenate([a, a])[:, None]
    lin1t = kw["lin1_w"].T                               # [25, 64]
    w["lin1_wT"] = np.zeros((32 + 25, D), np.float32)
    w["lin1_wT"][:25] = lin1t
    w["lin1_wT"][32:32 + 25] = lin1t
    w["lin1_b"] = col(kw["lin1_b"])                      # [128, 1]
    w["gate_w1aT"] = stack(kw["gate_lin1_w"][:, :D].T)   # [128, 64]
    w1b = kw["gate_lin1_w"][:, D:]                       # [64, 4]
    w["w1bRep"] = np.tile(w1b.T.reshape(-1)[None, :], (P, 1))   # [128, 4*64]
    w["attlRep"] = np.tile(kw["gate_att_l"][None, :], (P, 1))
    w["gateattrRep"] = np.tile(kw["gate_att_r"][None, :], (P, 1))
    w["gate_w2T"] = stack(kw["gate_lin2_w"].T)
    w["gate_bias"] = col(kw["gate_bias"])
    def gb(a):                      # [192] -> [128, 3]: [h*64+f, g] = a[g*64+f]
        t = a.reshape(3, D).T
        return np.concatenate([t, t], 0)
    for pre, g in [("gru0_", "gru0"), ("mol_gru_", "mol_gru")]:
        w[pre + "wih"] = stack(kw[g + "_wih"].T)         # [128, 192]
        w[pre + "whh"] = stack(kw[g + "_whh"].T)
        w[pre + "bih"] = gb(kw[g + "_bih"])              # [128, 3]
        w[pre + "bhh"] = gb(kw[g + "_bhh"])
        w[pre + "bsum"] = gb(kw[g + "_bih"] + kw[g + "_bhh"])
    for l in range(4):
        pre = f"at{l}_"
        w[pre + "wT"] = stack(kw["atom_lin_w"][l].T)
        w[pre + "srcRep"] = np.tile(kw["atom_att_src"][l][None, :], (P, 1))
        w[pre + "dstRep"] = np.tile(kw["atom_att_dst"][l][None, :], (P, 1))
        w[pre + "bias"] = col(kw["atom_bias"][l])
        w[pre + "gru_wih"] = stack(kw["atom_gru_wih"][l].T)
        w[pre + "gru_whh"] = stack(kw["atom_gru_whh"][l].T)
        w[pre + "gru_bih"] = gb(kw["atom_gru_bih"][l])
        w[pre + "gru_bhh"] = gb(kw["atom_gru_bhh"][l])
        w[pre + "gru_bsum"] = gb(kw["atom_gru_bih"][l] + kw["atom_gru_bhh"][l])
    w["mol_wT"] = stack(kw["mol_lin_w"].T)
    w["molsrcRep"] = np.tile(kw["mol_att_src"][None, :], (P, 1))
    w["moldstRep"] = np.tile(kw["mol_att_dst"][None, :], (P, 1))
    w["mol_biasRep"] = np.tile(kw["mol_bias"][None, :], (P, 1))
    w["lin2_wT"] = kw["lin2_w"].T.copy()                 # [64, 1]
    w["lin2_b"] = kw["lin2_b"][:, None].copy()           # [1, 1]
    return {k: np.ascontiguousarray(v, np.float32) for k, v in w.items()}


def _build(n_pad, t_tiles, tc_max, CH, wshapes):
    import concourse.bacc as bacc
    import concourse.mybir as mybir
    import concourse.tile as tile
    from concourse.bass import IndirectOffsetOnAxis
    from concourse.masks import make_identity

    dt = mybir.dt
    AF = mybir.ActivationFunctionType
    OP = mybir.AluOpType
    AX = mybir.AxisListType

    HC = n_pad // 2              # columns per half
    HT = t_tiles // 2            # node-tiles per half
    NCH = (HC + 511) // 512      # 512-col chunks per half
    GW = max(tc_max)             # max chunks per node-tile

    nc = bacc.Bacc("TRN2", target_bir_lowering=False, debug=False, num_devices=NC)

    xfm_h = nc.dram_tensor("xfm", [32 + F_IN, HC], dt.float32, kind="ExternalInput")
    idx_h = nc.dram_tensor("idx", [P, CH], dt.int32, kind="ExternalInput")
    rel_h = nc.dram_tensor("rel", [P, CH], dt.float32, kind="ExternalInput")
    attr_h = nc.dram_tensor("attr", [P, CH * E_DIM], dt.float32, kind="ExternalInput")
    grel0_h = nc.dram_tensor("grel0", [P, t_tiles], dt.float32, kind="ExternalInput")
    grel1_h = nc.dram_tensor("grel1", [P, t_tiles], dt.float32, kind="ExternalInput")
    iota_h = nc.dram_tensor("iotaRow", [P, P], dt.float32, kind="ExternalInput")
    cst_h = {k: nc.dram_tensor("w_" + k, list(s), dt.float32, kind="ExternalInput")
             for k, s in wshapes.items()}
    y_out = nc.dram_tensor("y", [1, G_LOC], dt.float32, kind="ExternalOutput")

    with tile.TileContext(nc) as tc:
      with (
        tc.tile_pool(name="cst", bufs=1) as cst,
        tc.tile_pool(name="st", bufs=1) as st,
        tc.tile_pool(name="ep", bufs=2) as ep,
        tc.tile_pool(name="sp", bufs=2) as sp,
        tc.tile_pool(name="ps", bufs=2, space="PSUM") as ps,
        tc.tile_pool(name="ps_seg", bufs=2, space="PSUM") as ps_seg,
        tc.tile_pool(name="ps_big", bufs=2, space="PSUM") as ps_big,
        tc.tile_pool(name="dram", bufs=1, space="DRAM") as dp,
      ):
        def load(name):
            h = cst_h[name]
            t = cst.tile(list(h.shape), dt.float32, name="c_" + name)
            nc.sync.dma_start(out=t[:], in_=h[:])
            return t
        W = {k: load(k) for k in cst_h}
        idx_sb = cst.tile([P, CH], dt.int32, name="idx_sb")
        nc.sync.dma_start(out=idx_sb[:], in_=idx_h[:])
        rel_sb = cst.tile([P, CH], dt.float32, name="rel_sb")
        nc.sync.dma_start(out=rel_sb[:], in_=rel_h[:])
        attr_sb = cst.tile([P, CH * E_DIM], dt.float32, name="attr_sb")
        nc.sync.dma_start(out=attr_sb[:], in_=attr_h[:])
        grel0_sb = cst.tile([P, t_tiles], dt.float32, name="grel0_sb")
        nc.sync.dma_start(out=grel0_sb[:], in_=grel0_h[:])
        grel1_sb = cst.tile([P, t_tiles], dt.float32, name="grel1_sb")
        nc.sync.dma_start(out=grel1_sb[:], in_=grel1_h[:])
        iota_sb = cst.tile([P, P], dt.float32, name="iota_sb")
        nc.sync.dma_start(out=iota_sb[:], in_=iota_h[:])
        ident = cst.tile([P, P], dt.float32, name="ident")
        make_identity(nc, ident[:])
        ones1 = cst.tile([1, P], dt.float32, name="ones1")
        nc.vector.memset(ones1[:], 1.0)

        XC = st.tile([P, HC], dt.float32, name="XC")    # node state (fm half-packed)
        HX = st.tile([P, HC], dt.float32, name="HX")    # xt / z / agg / h (fm)
        ad_nm = st.tile([P, t_tiles], dt.float32, name="ad_nm")   # alpha_dst per node
        tbl_locs = [dp.tile([n_pad, W1], dt.float32, name=f"tbl_loc{i}")
                    for i in range(5)]
        tbl_alls = [dp.tile([NC * n_pad, W1], dt.float32, addr_space="Shared",
                            name=f"tbl_all{i}") for i in range(5)]

        def halves(ap, h, c0, c1, k=D):
            return ap[h * k:(h + 1) * k, c0:c1]

        def mm_T(dst, wkey, src, act=AF.Copy, bias=None, alpha=0.0, kdim=D):
            """dst = act(W.T @ src + bias), feature-major half-packed."""
            for h in range(2):
                for ci in range(NCH):
                    c0, c1 = ci * 512, min((ci + 1) * 512, HC)
                    pt = ps_big.tile([P, 512], dt.float32, name="mmp", tag="mmp")
                    nc.tensor.matmul(halves(pt, h, 0, c1 - c0),
                                     W[wkey][h * kdim:(h + 1) * kdim, :],
                                     halves(src, h, c0, c1, kdim),
                                     start=True, stop=True)
                    b = W[bias][h * D:(h + 1) * D, 0:1] if bias else 0.0
                    nc.scalar.activation(halves(dst, h, c0, c1),
                                         halves(pt, h, 0, c1 - c0),
                                         act, bias=b, alpha=alpha)

        def elu_inplace(t_fm, bias):
            """t = elu(t + bias) in place, feature-major."""
            for h in range(2):
                for ci in range(NCH):
                    c0, c1 = ci * 512, min((ci + 1) * 512, HC)
                    w_ = c1 - c0
                    v = halves(t_fm, h, c0, c1)
                    tin = sp.tile([P, 512], dt.float32, name="eluin", tag="eluin")
                    nc.vector.tensor_scalar(out=halves(tin, h, 0, w_), in0=v,
                                            scalar1=W[bias][h * D:(h + 1) * D, 0:1],
                                            scalar2=None, op0=OP.add)
                    r = sp.tile([P, 512], dt.float32, name="elur", tag="elur")
                    nc.scalar.activation(halves(r, h, 0, w_), halves(tin, h, 0, w_),
                                         AF.Relu)
                    nc.vector.tensor_sub(halves(tin, h, 0, w_), halves(tin, h, 0, w_),
                                         halves(r, h, 0, w_))
                    nc.scalar.activation(halves(tin, h, 0, w_), halves(tin, h, 0, w_),
                                         AF.Exp)
                    nc.vector.scalar_tensor_tensor(
                        out=v, in0=halves(tin, h, 0, w_), scalar=-1.0,
                        in1=halves(r, h, 0, w_), op0=OP.add, op1=OP.add)

        def gru_relu(pre):
            """XC = relu(GRUCell(input=HX, hidden=XC)), in place."""
            for h in range(2):
                for ci in range(NCH):
                    c0, c1 = ci * 512, min((ci + 1) * 512, HC)
                    w_ = c1 - c0
                    wih = W[pre + "wih"]
                    whh = W[pre + "whh"]
                    def gate2(g):
                        pt = ps_big.tile([P, 512], dt.float32, name="grup", tag="mmp")
                        nc.tensor.matmul(halves(pt, h, 0, w_),
                                         wih[h * D:(h + 1) * D, g * D:(g + 1) * D],
                                         halves(HX, h, c0, c1), start=True, stop=False)
                        nc.tensor.matmul(halves(pt, h, 0, w_),
                                         whh[h * D:(h + 1) * D, g * D:(g + 1) * D],
                                         halves(XC, h, c0, c1), start=False, stop=True)
                        return pt
                    bs = W[pre + "bsum"]
                    pr = gate2(0)
                    r = sp.tile([P, 512], dt.float32, name="grur", tag="grur")
                    nc.scalar.activation(halves(r, h, 0, w_), halves(pr, h, 0, w_),
                                         AF.Sigmoid, bias=bs[h * D:(h + 1) * D, 0:1])
                    pz = gate2(1)
                    z = sp.tile([P, 512], dt.float32, name="gruz", tag="gruz")
                    nc.scalar.activation(halves(z, h, 0, w_), halves(pz, h, 0, w_),
                                         AF.Sigmoid,
                                         bias=bs[h * D:(h + 1) * D, 1:2])
                    pin = ps_big.tile([P, 512], dt.float32, name="grupi", tag="mmp")
                    nc.tensor.matmul(halves(pin, h, 0, w_),
                                     wih[h * D:(h + 1) * D, 2 * D:3 * D],
                                     halves(HX, h, c0, c1), start=True, stop=True)
                    phn = ps_big.tile([P, 512], dt.float32, name="gruph", tag="mmp")
                    nc.tensor.matmul(halves(phn, h, 0, w_),
                                     whh[h * D:(h + 1) * D, 2 * D:3 * D],
                                     halves(XC, h, c0, c1), start=True, stop=True)
                    hn = sp.tile([P, 512], dt.float32, name="gruhn", tag="gruhn")
                    nc.vector.tensor_scalar(
                        out=halves(hn, h, 0, w_), in0=halves(phn, h, 0, w_),
                        scalar1=W[pre + "bhh"][h * D:(h + 1) * D, 2:3],
                        scalar2=None, op0=OP.add)
                    nc.vector.tensor_mul(halves(hn, h, 0, w_), halves(hn, h, 0, w_),
                                         halves(r, h, 0, w_))
                    nc.vector.tensor_tensor(out=halves(hn, h, 0, w_),
                                            in0=halves(hn, h, 0, w_),
                                            in1=halves(pin, h, 0, w_), op=OP.add)
                    n_t = sp.tile([P, 512], dt.float32, name="grun", tag="grun")
                    nc.scalar.activation(
                        halves(n_t, h, 0, w_), halves(hn, h, 0, w_), AF.Tanh,
                        bias=W[pre + "bih"][h * D:(h + 1) * D, 2:3])
                    d_t = sp.tile([P, 512], dt.float32, name="grud", tag="grud")
                    nc.vector.tensor_sub(halves(d_t, h, 0, w_), halves(XC, h, c0, c1),
                                         halves(n_t, h, 0, w_))
                    nc.vector.tensor_mul(halves(d_t, h, 0, w_), halves(d_t, h, 0, w_),
                                         halves(z, h, 0, w_))
                    nc.vector.tensor_tensor(out=halves(d_t, h, 0, w_),
                                            in0=halves(d_t, h, 0, w_),
                                            in1=halves(n_t, h, 0, w_), op=OP.add)
                    nc.scalar.activation(halves(XC, h, c0, c1), halves(d_t, h, 0, w_),
                                         AF.Relu)

        def tile_fm_slice(t_fm, t):
            h = t // HT
            cc = (t % HT) * P
            return t_fm[h * D:(h + 1) * D, cc:cc + P], ident[h * D:(h + 1) * D,
                                                             h * D:(h + 1) * D]

        def build_table(src_fm, srcRep, dstRep, li):
            """Write tbl_loc rows [xt | alpha_src] + ad_nm (alpha_dst), allgather."""
            tbl_loc = tbl_locs[li]
            tbl_all = tbl_alls[li]
            for t in range(t_tiles):
                sl, idn = tile_fm_slice(src_fm, t)
                pt = ps.tile([P, D], dt.float32, name="tabT", tag="miscP")
                nc.tensor.transpose(out=pt[:], in_=sl, identity=idn)
                row = sp.tile([P, W1], dt.float32, name="row", tag="row")
                nc.vector.tensor_copy(out=row[:, 0:D], in_=pt[:])
                tmp = sp.tile([P, D], dt.float32, name="rowt", tag="rowt")
                if srcRep is None:
                    nc.vector.memset(row[:, D:W1], 0.0)
                else:
                    nc.vector.tensor_tensor(out=tmp[:], in0=row[:, 0:D],
                                            in1=W[srcRep][:], op=OP.mult)
                    nc.vector.tensor_reduce(row[:, D:W1], tmp[:], axis=AX.X, op=OP.add)
                nc.vector.tensor_tensor(out=tmp[:], in0=row[:, 0:D],
                                        in1=W[dstRep][:], op=OP.mult)
                nc.vector.tensor_reduce(ad_nm[:, t:t + 1], tmp[:], axis=AX.X, op=OP.add)
                nc.sync.dma_start(out=tbl_loc[t * P:(t + 1) * P, :], in_=row[:])
            nc.gpsimd.collective_compute(
                "AllGather", mybir.AluOpType.bypass,
                replica_groups=[list(range(NC))],
                ins=[tbl_loc.opt()], outs=[tbl_all.opt()])

        def edge_phase(is_gate, li):
            """Message passing round; writes agg (normalized) into HX (fm)."""
            tbl_all = tbl_alls[li]
            ch0 = 0
            for t in range(t_tiles):
                nch = tc_max[t]
                gt = ep.tile([P, GW * W1], dt.float32, name="gt", tag="gt", bufs=3)
                for i in range(nch):
                    nc.gpsimd.indirect_dma_start(
                        out=gt[:, i * W1:(i + 1) * W1], out_offset=None,
                        in_=tbl_all[:],
                        in_offset=IndirectOffsetOnAxis(
                            ap=idx_sb[:, ch0 + i:ch0 + i + 1], axis=0))
                # alpha_dst replicated across partitions for this tile
                pa = ps.tile([1, P], dt.float32, name="adT", tag="rowP")
                nc.tensor.transpose(out=pa[:], in_=ad_nm[:, t:t + 1],
                                    identity=ident[:])
                arT = sp.tile([1, P], dt.float32, name="arT", tag="arT")
                nc.vector.tensor_copy(out=arT[:], in_=pa[:])
                pr = ps.tile([P, P], dt.float32, name="repP", tag="miscP")
                nc.tensor.matmul(pr[:], ones1[:], arT[:], start=True, stop=True)
                rep = sp.tile([P, P], dt.float32, name="rep", tag="rep")
                nc.vector.tensor_copy(out=rep[:], in_=pr[:])
                # one-hot S for all chunks: [128, nch*128]
                s_all = ep.tile([P, GW * P], dt.float32, name="s_all", tag="s_all")
                nc.vector.tensor_tensor(
                    out=s_all[:, :nch * P],
                    in0=rel_sb[:, ch0:ch0 + nch].unsqueeze(2).to_broadcast([P, nch, P]),
                    in1=iota_sb[:].unsqueeze(1).to_broadcast([P, nch, P]),
                    op=OP.is_equal)
                # alpha_dst per slot
                sa = ep.tile([P, GW * P], dt.float32, name="sa", tag="sa", bufs=1)
                nc.vector.tensor_tensor(
                    out=sa[:, :nch * P], in0=s_all[:, :nch * P],
                    in1=rep[:].unsqueeze(1).to_broadcast([P, nch, P]), op=OP.mult)
                a_slot = sp.tile([P, GW], dt.float32, name="a_slot", tag="a_slot")
                nc.vector.tensor_reduce(a_slot[:, 0:nch].unsqueeze(2),
                                        sa[:, :nch * P].rearrange(
                                            "p (c q) -> p c q", q=P),
                                        axis=AX.X, op=OP.add)
                gt3 = gt[:].rearrange("p (c w) -> p c w", w=W1)
                if is_gate:
                    m_all = ep.tile([P, GW * D], dt.float32, name="m_all", tag="m_all")
                    m3 = m_all[:].rearrange("p (c w) -> p c w", w=D)
                    nc.vector.tensor_copy(out=m3[:, 0:nch, :], in_=gt3[:, 0:nch, 0:D])
                    at3 = attr_sb[:].rearrange("p (c e) -> p c e", e=E_DIM)
                    for j in range(E_DIM):
                        tj = ep.tile([P, GW * D], dt.float32, name="tj", tag="tj",
                                     bufs=1)
                        nc.vector.tensor_tensor(
                            out=tj[:, :nch * D],
                            in0=at3[:, ch0:ch0 + nch, j:j + 1].to_broadcast(
                                [P, nch, D]),
                            in1=W["w1bRep"][:, j * D:(j + 1) * D].unsqueeze(1)
                                .to_broadcast([P, nch, D]),
                            op=OP.mult)
                        nc.vector.tensor_tensor(out=m_all[:, :nch * D],
                                                in0=m_all[:, :nch * D],
                                                in1=tj[:, :nch * D], op=OP.add)
                    nc.scalar.activation(m_all[:, :nch * D], m_all[:, :nch * D],
                                         AF.Prelu, alpha=NEG)
                    lt = ep.tile([P, GW * D], dt.float32, name="lt", tag="tj", bufs=1)
                    nc.vector.tensor_tensor(
                        out=lt[:, :nch * D], in0=m_all[:, :nch * D],
                        in1=W["attlRep"][:].unsqueeze(1).to_broadcast([P, nch, D]),
                        op=OP.mult)
                    q = sp.tile([P, GW], dt.float32, name="q", tag="q")
                    nc.vector.tensor_reduce(q[:, 0:nch].unsqueeze(2),
                                            lt[:, :nch * D].rearrange(
                                                "p (c w) -> p c w", w=D),
                                            axis=AX.X, op=OP.add)
                    nc.vector.tensor_tensor(out=q[:, 0:nch], in0=q[:, 0:nch],
                                            in1=a_slot[:, 0:nch], op=OP.add)
                    msg3 = m3
                else:
                    q = sp.tile([P, GW], dt.float32, name="q", tag="q")
                    nc.vector.tensor_tensor(out=q[:, 0:nch], in0=gt3[:, 0:nch, D:W1].squeeze(2),
                                            in1=a_slot[:, 0:nch], op=OP.add)
                    msg3 = gt3
                e_t = sp.tile([P, GW], dt.float32, name="e_t", tag="e_t")
                nc.scalar.activation(e_t[:, 0:nch], q[:, 0:nch], AF.Prelu, alpha=NEG)
                nc.scalar.activation(e_t[:, 0:nch], e_t[:, 0:nch], AF.Exp)
                rhs = ep.tile([P, GW * W1], dt.float32, name="rhs", tag="rhs")
                r3 = rhs[:].rearrange("p (c w) -> p c w", w=W1)
                nc.vector.tensor_tensor(
                    out=r3[:, 0:nch, 0:D], in0=msg3[:, 0:nch, 0:D],
                    in1=e_t[:, 0:nch].unsqueeze(2).to_broadcast([P, nch, D]),
                    op=OP.mult)
                nc.vector.tensor_copy(out=r3[:, 0:nch, D:W1],
                                      in_=e_t[:, 0:nch].unsqueeze(2))
                pseg = ps_seg.tile([P, W1], dt.float32, name="pseg", tag="pseg")
                for i in range(nch):
                    nc.tensor.matmul(pseg[:], s_all[:, i * P:(i + 1) * P],
                                     rhs[:, i * W1:(i + 1) * W1],
                                     start=(i == 0), stop=(i == nch - 1))
                sn = sp.tile([P, 1], dt.float32, name="sn", tag="sn")
                nc.vector.tensor_single_scalar(out=sn[:], in_=pseg[:, D:W1],
                                               scalar=1e-16, op=OP.add)
                rcp = sp.tile([P, 1], dt.float32, name="rcp", tag="rcp")
                nc.vector.reciprocal(rcp[:], sn[:])
                agg = sp.tile([P, D], dt.float32, name="agg", tag="agg")
                nc.vector.tensor_tensor(out=agg[:], in0=pseg[:, 0:D],
                                        in1=rcp[:].to_broadcast([P, D]), op=OP.mult)
                sl, idn = tile_fm_slice(HX, t)
                h = t // HT
                pT = ps.tile([P, P], dt.float32, name="aggT", tag="miscP")
                nc.tensor.matmul(pT[h * D:(h + 1) * D, :], agg[:], ident[:],
                                 start=True, stop=True)
                nc.vector.tensor_copy(out=sl, in_=pT[h * D:(h + 1) * D, 0:P])
                ch0 += nch

        # ================= forward =================
        # xh = lrelu(x @ lin1.T + b) -> XC  (x streamed per chunk)
        for h in range(2):
            for ci in range(NCH):
                c0, c1 = ci * 512, min((ci + 1) * 512, HC)
                xin = sp.tile([32 + F_IN, 512], dt.float32, name="xin", tag="xin")
                nc.sync.dma_start(out=xin[:, :c1 - c0], in_=xfm_h[:, c0:c1])
                pt = ps_big.tile([P, 512], dt.float32, name="mmp0", tag="mmp")
                nc.tensor.matmul(halves(pt, h, 0, c1 - c0),
                                 W["lin1_wT"][h * 32:h * 32 + F_IN, :],
                                 xin[h * 32:h * 32 + F_IN, c0 - c0:c1 - c0],
                                 start=True, stop=True)
                nc.scalar.activation(halves(XC, h, c0, c1), halves(pt, h, 0, c1 - c0),
                                     AF.Prelu, bias=W["lin1_b"][h * D:(h + 1) * D, 0:1],
                                     alpha=NEG)
        # --- GATEConv ---
        mm_T(HX, "gate_w1aT", XC)                       # z
        build_table(HX, None, "gateattrRep", 0)         # table=[z|0]
        # NOTE: alpha_r must come from xh (XC), not z: recompute ad_nm from XC
        for t in range(t_tiles):
            sl, idn = tile_fm_slice(XC, t)
            pt = ps.tile([P, D], dt.float32, name="adfix", tag="miscP")
            nc.tensor.transpose(out=pt[:], in_=sl, identity=idn)
            tmp = sp.tile([P, D], dt.float32, name="adfixt", tag="rowt")
            nc.vector.tensor_tensor(out=tmp[:], in0=pt[:], in1=W["gateattrRep"][:],
                                    op=OP.mult)
            nc.vector.tensor_reduce(ad_nm[:, t:t + 1], tmp[:], axis=AX.X, op=OP.add)
        edge_phase(True, 0)
        mm_T(HX, "gate_w2T", HX)                        # W2 @ agg (in place)
        elu_inplace(HX, "gate_bias")
        gru_relu("gru0_")
        # --- atom layers ---
        for l in range(4):
            pre = f"at{l}_"
            mm_T(HX, pre + "wT", XC)                    # xt
            build_table(HX, pre + "srcRep", pre + "dstRep", 1 + l)
            edge_phase(False, 1 + l)
            elu_inplace(HX, pre + "bias")
            gru_relu(pre + "gru_")

        # ================= readout =================
        xc_nm = st.tile([P, t_tiles * D], dt.float32, name="xc_nm", tag="HX")
        for t in range(t_tiles):
            sl, idn = tile_fm_slice(XC, t)
            pt = ps.tile([P, D], dt.float32, name="xcT", tag="miscP")
            nc.tensor.transpose(out=pt[:], in_=sl, identity=idn)
            nc.vector.tensor_copy(out=xc_nm[:, t * D:(t + 1) * D], in_=pt[:])
        OUTT = st.tile([D, 2 * P], dt.float32, name="OUTT")
        sgk = []
        for k, grel in ((0, grel0_sb), (1, grel1_sb)):
            pg = ps_seg.tile([P, D], dt.float32, name="pg", tag="pseg")
            first = True
            for t in range(t_tiles):
                sg = sp.tile([P, P], dt.float32, name=f"sg{k}", tag="sg")
                nc.vector.tensor_tensor(
                    out=sg[:], in0=grel[:, t:t + 1].to_broadcast([P, P]),
                    in1=iota_sb[:], op=OP.is_equal)
                nc.tensor.matmul(pg[:], sg[:], xc_nm[:, t * D:(t + 1) * D],
                                 start=(t == 0), stop=(t == t_tiles - 1))
            og = sp.tile([P, D], dt.float32, name="og", tag="agg")
            nc.scalar.activation(og[:], pg[:], AF.Relu)
            pT = ps.tile([D, P], dt.float32, name="ogT", tag="miscP")
            nc.tensor.transpose(out=pT[:], in_=og[:], identity=ident[:])
            nc.vector.tensor_copy(out=OUTT[:, k * P:(k + 1) * P], in_=pT[:])
        # xs (fm), then node-major xs + a_src
        XS = st.tile([P, HC], dt.float32, name="XS")
        mm_T(XS, "mol_wT", XC)
        xs_nm = st.tile([P, t_tiles * D], dt.float32, name="xs_nm", tag="XC")
        asrc_nm = st.tile([P, t_tiles], dt.float32, name="asrc_nm")
        for t in range(t_tiles):
            sl, idn = tile_fm_slice(XS, t)
            pt = ps.tile([P, D], dt.float32, name="xsT", tag="miscP")
            nc.tensor.transpose(out=pt[:], in_=sl, identity=idn)
            nc.vector.tensor_copy(out=xs_nm[:, t * D:(t + 1) * D], in_=pt[:])
            tmp = sp.tile([P, D], dt.float32, name="xst", tag="rowt")
            nc.vector.tensor_tensor(out=tmp[:], in0=pt[:], in1=W["molsrcRep"][:],
                                    op=OP.mult)
            nc.vector.tensor_reduce(asrc_nm[:, t:t + 1], tmp[:], axis=AX.X, op=OP.add)
        HG = st.tile([D, 2 * P], dt.float32, name="HG")
        for ts in range(3):
            # xd = mol_w @ out; ag[g] = att_dst . xd
            pt = ps_big.tile([D, 512], dt.float32, name="xdp", tag="mmp")
            nc.tensor.matmul(pt[:, 0:2 * P], W["mol_wT"][0:D, :], OUTT[:],
                             start=True, stop=True)
            xd_nm0 = sp.tile([P, D], dt.float32, name="xdnm0", tag="xdnm0")
            xd_nm1 = sp.tile([P, D], dt.float32, name="xdnm1", tag="xdnm1")
            for k, xd_nm in ((0, xd_nm0), (1, xd_nm1)):
                pT = ps.tile([P, D], dt.float32, name="xdT", tag="miscP")
                xds = sp.tile([D, P], dt.float32, name="xds", tag="xds")
                nc.vector.tensor_copy(out=xds[:], in_=pt[:, k * P:(k + 1) * P])
                nc.tensor.transpose(out=pT[:], in_=xds[:], identity=ident[0:D, 0:D])
                nc.vector.tensor_copy(out=xd_nm[:], in_=pT[:])
            agr = sp.tile([P, 2], dt.float32, name="agr", tag="agr")
            for k, xd_nm in ((0, xd_nm0), (1, xd_nm1)):
                tmp = sp.tile([P, D], dt.float32, name="agt", tag="rowt")
                nc.vector.tensor_tensor(out=tmp[:], in0=xd_nm[:],
                                        in1=W["moldstRep"][:], op=OP.mult)
                nc.vector.tensor_reduce(agr[:, k:k + 1], tmp[:], axis=AX.X, op=OP.add)
            psg = [ps_seg.tile([P, W1], dt.float32, name=f"psg{k}", tag="pseg")
                   for k in range(2)]
            for t in range(t_tiles):
                # a_node = sum_k reduce(sg_k * rep(agr_k))
                a_n = sp.tile([P, 1], dt.float32, name="a_n", tag="a_n")
                nc.vector.memset(a_n[:], 0.0)
                sgs = []
                for k, grel in ((0, grel0_sb), (1, grel1_sb)):
                    sg = sp.tile([P, P], dt.float32, name=f"sgr{k}", tag=f"sgr{k}")
                    nc.vector.tensor_tensor(
                        out=sg[:], in0=grel[:, t:t + 1].to_broadcast([P, P]),
                        in1=iota_sb[:], op=OP.is_equal)
                    sgs.append(sg)
                    pa = ps.tile([1, P], dt.float32, name="agrT", tag="rowP")
                    nc.tensor.transpose(out=pa[:], in_=agr[:, k:k + 1],
                                        identity=ident[:])
                    arT = sp.tile([1, P], dt.float32, name="agrTs", tag="arT")
                    nc.vector.tensor_copy(out=arT[:], in_=pa[:])
                    prr = ps.tile([P, P], dt.float32, name="repPg", tag="miscP")
                    nc.tensor.matmul(prr[:], ones1[:], arT[:], start=True, stop=True)
                    tmp = sp.tile([P, P], dt.float32, name="sgt", tag="sg")
                    nc.vector.tensor_tensor(out=tmp[:], in0=sgs[k][:], in1=prr[:],
                                            op=OP.mult)
                    red = sp.tile([P, 1], dt.float32, name="red", tag="red")
                    nc.vector.tensor_reduce(red[:], tmp[:], axis=AX.X, op=OP.add)
                    nc.vector.tensor_tensor(out=a_n[:], in0=a_n[:], in1=red[:],
                                            op=OP.add)
                q_n = sp.tile([P, 1], dt.float32, name="q_n", tag="q_n")
                nc.vector.tensor_tensor(out=q_n[:], in0=asrc_nm[:, t:t + 1],
                                        in1=a_n[:], op=OP.add)
                e_n = sp.tile([P, 1], dt.float32, name="e_n", tag="e_n")
                nc.scalar.activation(e_n[:], q_n[:], AF.Prelu, alpha=NEG)
                nc.scalar.activation(e_n[:], e_n[:], AF.Exp)
                rh = sp.tile([P, W1], dt.float32, name="rh", tag="rh")
                nc.vector.tensor_tensor(out=rh[:, 0:D],
                                        in0=xs_nm[:, t * D:(t + 1) * D],
                                        in1=e_n[:].to_broadcast([P, D]), op=OP.mult)
                nc.vector.tensor_copy(out=rh[:, D:W1], in_=e_n[:])
                for k in range(2):
                    nc.tensor.matmul(psg[k][:], sgs[k][:], rh[:],
                                     start=(t == 0), stop=(t == t_tiles - 1))
            for k in range(2):
                sn = sp.tile([P, 1], dt.float32, name="sng", tag="sn")
                nc.vector.tensor_single_scalar(out=sn[:], in_=psg[k][:, D:W1],
                                               scalar=1e-16, op=OP.add)
                rcp = sp.tile([P, 1], dt.float32, name="rcpg", tag="rcp")
                nc.vector.reciprocal(rcp[:], sn[:])
                aggg = sp.tile([P, D], dt.float32, name="aggg", tag="agg")
                nc.vector.tensor_tensor(out=aggg[:], in0=psg[k][:, 0:D],
                                        in1=rcp[:].to_broadcast([P, D]), op=OP.mult)
                nc.vector.tensor_tensor(out=aggg[:], in0=aggg[:],
                                        in1=W["mol_biasRep"][:], op=OP.add)
                r = sp.tile([P, D], dt.float32, name="rg", tag="rg")
                nc.scalar.activation(r[:], aggg[:], AF.Relu)
                xm = sp.tile([P, D], dt.float32, name="xmg", tag="xmg")
                nc.vector.tensor_sub(xm[:], aggg[:], r[:])
                nc.scalar.activation(xm[:], xm[:], AF.Exp)
                nc.vector.scalar_tensor_tensor(out=aggg[:], in0=xm[:], scalar=-1.0,
                                               in1=r[:], op0=OP.add, op1=OP.add)
                pT = ps.tile([D, P], dt.float32, name="hgT", tag="miscP")
                nc.tensor.transpose(out=pT[:], in_=aggg[:], identity=ident[:])
                nc.vector.tensor_copy(out=HG[:, k * P:(k + 1) * P], in_=pT[:])
            # graph GRU (feature-major, 256 cols, first half rows only)
            wih = W["mol_gru_wih"]
            whh = W["mol_gru_whh"]
            bs = W["mol_gru_bsum"]
            def gate2g(g):
                pt = ps_big.tile([D, 512], dt.float32, name="ggp", tag="mmp")
                nc.tensor.matmul(pt[:, 0:2 * P], wih[0:D, g * D:(g + 1) * D], HG[:],
                                 start=True, stop=False)
                nc.tensor.matmul(pt[:, 0:2 * P], whh[0:D, g * D:(g + 1) * D], OUTT[:],
                                 start=False, stop=True)
                return pt
            prg = gate2g(0)
            rg2 = sp.tile([D, 2 * P], dt.float32, name="ggr", tag="ggr")
            nc.scalar.activation(rg2[:], prg[:, 0:2 * P], AF.Sigmoid, bias=bs[0:D, 0:1])
            pzg = gate2g(1)
            zg = sp.tile([D, 2 * P], dt.float32, name="ggz", tag="ggz")
            nc.scalar.activation(zg[:], pzg[:, 0:2 * P], AF.Sigmoid,
                                 bias=bs[0:D, 1:2])
            pig = ps_big.tile([D, 512], dt.float32, name="ggpi", tag="mmp")
            nc.tensor.matmul(pig[:, 0:2 * P], wih[0:D, 2 * D:3 * D], HG[:],
                             start=True, stop=True)
            phg = ps_big.tile([D, 512], dt.float32, name="ggph", tag="mmp")
            nc.tensor.matmul(phg[:, 0:2 * P], whh[0:D, 2 * D:3 * D], OUTT[:],
                             start=True, stop=True)
            hng = sp.tile([D, 2 * P], dt.float32, name="gghn", tag="gghn")
            nc.vector.tensor_scalar(out=hng[:], in0=phg[:, 0:2 * P],
                                    scalar1=W["mol_gru_bhh"][0:D, 2:3],
                                    scalar2=None, op0=OP.add)
            nc.vector.tensor_mul(hng[:], hng[:], rg2[:])
            nc.vector.tensor_tensor(out=hng[:], in0=hng[:], in1=pig[:, 0:2 * P],
                                    op=OP.add)
            ng = sp.tile([D, 2 * P], dt.float32, name="ggn", tag="ggn")
            nc.scalar.activation(ng[:], hng[:], AF.Tanh,
                                 bias=W["mol_gru_bih"][0:D, 2:3])
            dg = sp.tile([D, 2 * P], dt.float32, name="ggd", tag="ggd")
            nc.vector.tensor_sub(dg[:], OUTT[:], ng[:])
            nc.vector.tensor_mul(dg[:], dg[:], zg[:])
            nc.vector.tensor_tensor(out=dg[:], in0=dg[:], in1=ng[:], op=OP.add)
            nc.scalar.activation(OUTT[:], dg[:], AF.Relu)
        # y = out @ lin2.T + b
        py = ps.tile([1, 2 * P], dt.float32, name="py", tag="rowP")
        nc.tensor.matmul(py[:], W["lin2_wT"][:], OUTT[:], start=True, stop=True)
        ysb = sp.tile([1, 2 * P], dt.float32, name="ysb", tag="ysb")
        nc.vector.tensor_scalar(out=ysb[:], in0=py[:], scalar1=W["lin2_b"][0:1, 0:1],
                                scalar2=None, op0=OP.add)
        nc.sync.dma_start(out=y_out[:], in_=ysb[0:1, 0:G_LOC])
    nc.compile()
    return nc


_CACHE = {}


def kernel(**inputs):
    from concourse.bass_utils import run_bass_kernel_spmd
    x = np.asarray(inputs["x"], np.float32)
    ei = np.asarray(inputs["edge_index"])
    ea = np.asarray(inputs["edge_attr"], np.float32)
    bt = np.asarray(inputs["batch"])
    per, n_pad, t_tiles, tc_max, CH = _prep(x, ei, ea, bt)
    weights = _mk_weights({k: np.asarray(v, np.float32) for k, v in inputs.items()
                           if k not in ("x", "edge_index", "edge_attr", "batch")})
    key = (n_pad, CH, tuple(tc_max))
    if key not in _CACHE:
        _CACHE[key] = _build(n_pad, t_tiles, tc_max, CH,
                             {k: v.shape for k, v in weights.items()})
    nc = _CACHE[key]
    iota = np.tile(np.arange(P, dtype=np.float32)[None, :], (P, 1))
    in_maps = []
    for c in range(NC):
        m = dict(xfm=per[c]["xfm"], idx=per[c]["idx"], rel=per[c]["rel"],
                 attr=per[c]["attr"], grel0=per[c]["grel0"], grel1=per[c]["grel1"],
                 iotaRow=iota)
        for k, v in weights.items():
            m["w_" + k] = v
        in_maps.append(m)
    res = run_bass_kernel_spmd(nc, in_maps, core_ids=list(range(NC)))
    return np.concatenate([res.results[c]["y"][0] for c in range(NC)]).astype(np.float32)



# revision 5
# speedup vs baseline: 3.0214x; 3.0214x over previous
"""AttentiveFP forward on 8 Trainium2 NeuronCores (Bass/Tile).

Sharding: 2048 graphs (nodes contiguous, batch sorted) split into 8 blocks of
256 graphs; each core owns the edges whose dst node falls in its block. Per
round each core computes its nodes' features, all-gathers two per-node tables
across the 8 cores — [xt | alpha_src] (bf16, 65 wide) and alpha_dst
([n_pad, 1] bf16) — then for its local edges:

- per-edge src rows fetched with BATCHED indirect DMA (one call per node-tile
  pair, offset AP [128, chunks]); per-edge alpha_dst fetched with 4 interleaved
  width-1 indirect gathers (HW indirect semantics: each descriptor moves
  `row_width` elements from idx*row_width+element_offset into the out AP)
- segment softmax + weighted segment sums are PSUM-accumulated one-hot
  matmuls in bf16; one-hots from is_equal(rel, iota) on DVE
- node phases run feature-major, nodes half-packed on the 128 partitions;
  node matmuls use block-diagonal [128,128] bf16 weights so one matmul
  covers both halves; GRU hidden state is kept fp32
- readout gathers per-node graph-alpha from a [256,1] DRAM table per timestep

Softmax max-subtraction is skipped (logits O(1), shift-invariant).
"""
import sys
sys.path.insert(0, '/opt/trn_rl_repo')
sys.path.insert(0, '/root/.axon_site')

import numpy as np
import ml_dtypes

BF16 = ml_dtypes.bfloat16
NC = 8
D = 64
G_TOT = 2048
G_LOC = G_TOT // NC
F_IN = 25
E_DIM = 4
NEG = 0.01
P = 128
W = 65          # gather-table row width: [xt(64) | alpha_src]


def _prep(x, edge_index, edge_attr, batch):
    src = edge_index[0].astype(np.int64)
    dst = edge_index[1].astype(np.int64)
    batch = batch.astype(np.int64)

    gstart = np.searchsorted(batch, np.arange(0, G_TOT + 1, G_LOC))
    n0 = gstart[:-1]
    nloc = np.diff(gstart)
    n_pad = int(np.ceil((nloc.max() + 1) / 256) * 256)
    t_tiles = n_pad // P
    HC = n_pad // 2
    HT = t_tiles // 2

    def pi_row(n):
        h = n // HC
        r = n % HC
        return (2 * (r // P) + h) * P + (r % P)

    src_dev = np.searchsorted(gstart[1:], src, side='right')
    dst_dev = np.searchsorted(gstart[1:], dst, side='right')
    gidx_all = src_dev * n_pad + pi_row(src - n0[src_dev])

    per = []
    for c in range(NC):
        sel = np.where(dst_dev == c)[0]
        dl = dst[sel] - n0[c]
        j_dst = 2 * ((dl % HC) // P) + dl // HC
        p_dst = dl % P
        order = np.argsort(j_dst, kind='stable')
        sel, j_dst, p_dst = sel[order], j_dst[order], p_dst[order]
        gi = gidx_all[sel]
        gd = c * n_pad + j_dst * P + p_dst
        at = edge_attr[sel].astype(np.float32)
        idx_cols, idxd_cols, rel_cols, attr_cols, tile_chunks = [], [], [], [], []
        for j in range(t_tiles):
            m = j_dst == j
            k = int(m.sum())
            nch = max(1, (k + P - 1) // P)
            pad = nch * P - k
            gi_j = np.concatenate([gi[m], np.zeros(pad, np.int64)])
            gd_j = np.concatenate([gd[m], np.zeros(pad, np.int64)])
            rel_j = np.concatenate([p_dst[m].astype(np.float32),
                                    np.full(pad, 255.0, np.float32)])
            at_j = np.concatenate([at[m], np.zeros((pad, E_DIM), np.float32)], 0)
            idx_cols.append(gi_j.reshape(nch, P).T)
            idxd_cols.append(gd_j.reshape(nch, P).T)
            rel_cols.append(rel_j.reshape(nch, P).T)
            attr_cols.append(at_j.reshape(nch, P, E_DIM).transpose(1, 0, 2)
                             .reshape(P, nch * E_DIM))
            tile_chunks.append(nch)
        per.append(dict(idx_cols=idx_cols, idxd_cols=idxd_cols, rel_cols=rel_cols,
                        attr_cols=attr_cols, tile_chunks=tile_chunks))
        nl = int(nloc[c])
        gl = batch[n0[c]:n0[c] + nl] - G_LOC * c
        grel = np.full((P, t_tiles, 2), 300.0, np.float32)
        idxg = np.zeros((P, t_tiles), np.int32)
        n_ids = np.arange(n_pad)
        h_a = n_ids // HC
        tp_a = (n_ids % HC) // P
        p_a = n_ids % P
        j_a = 2 * tp_a + h_a
        valid = n_ids < nl
        for kk in range(2):
            grel[p_a[valid], j_a[valid], kk] = gl[n_ids[valid]] - kk * P
        idxg[p_a[valid], j_a[valid]] = gl[n_ids[valid]]
        per[c]['grel0'] = np.ascontiguousarray(grel[:, :, 0].astype(BF16))
        per[c]['grel1'] = np.ascontiguousarray(grel[:, :, 1].astype(BF16))
        per[c]['idxg'] = idxg
        xp = np.zeros((n_pad, F_IN), np.float32)
        xp[:nl] = x[n0[c]:n0[c] + nl]
        xfm = np.zeros((P, HC), np.float32)
        xfm[:F_IN] = xp[:HC].T
        xfm[D:D + F_IN] = xp[HC:].T
        per[c]['xfm'] = xfm.astype(BF16)
    tc_max = [max(per[c]['tile_chunks'][j] for c in range(NC)) for j in range(t_tiles)]
    CH = sum(tc_max)
    for c in range(NC):
        idx_n = np.zeros((P, CH), np.int32)
        idxd_n = np.zeros((P, CH), np.int32)
        rel_n = np.full((P, CH), 255.0, np.float32)
        attr_n = np.zeros((P, CH * E_DIM), np.float32)
        s = 0
        for j in range(t_tiles):
            k = per[c]['tile_chunks'][j]
            idx_n[:, s:s + k] = per[c]['idx_cols'][j]
            idxd_n[:, s:s + k] = per[c]['idxd_cols'][j]
            rel_n[:, s:s + k] = per[c]['rel_cols'][j]
            attr_n[:, s * E_DIM:(s + k) * E_DIM] = per[c]['attr_cols'][j]
            s += tc_max[j]
        per[c]['idx'] = idx_n
        per[c]['idxd'] = idxd_n
        per[c]['rel'] = rel_n.astype(BF16)
        per[c]['attr'] = attr_n.astype(BF16)
        for k in ('idx_cols', 'idxd_cols', 'rel_cols', 'attr_cols'):
            del per[c][k]
    return per, n_pad, t_tiles, tc_max, CH


def _mk_weights(kw):
    w = {}
    def bd(a):
        t = a.T
        z = np.zeros((P, P), np.float32)
        z[0:D, 0:D] = t
        z[D:2 * D, D:2 * D] = t
        return z
    def col(a):
        return np.concatenate([a, a])[:, None]
    def rep2(a):
        return np.tile(np.concatenate([a, a])[None, :], (P, 1))
    def rep1(a):
        return np.tile(a[None, :], (P, 1))
    def gb(a):
        t = a.reshape(3, D).T
        return np.concatenate([t, t], 0)
    def gru_bd(wg):
        out = np.zeros((P, 3 * P), np.float32)
        for g in range(3):
            out[:, g * P:(g + 1) * P] = bd(wg[g * D:(g + 1) * D])
        return out

    B, F = 'b', 'f'
    lin1 = np.zeros((P, P), np.float32)
    lin1[0:F_IN, 0:D] = kw["lin1_w"].T
    lin1[D:D + F_IN, D:2 * D] = kw["lin1_w"].T
    w["lin1_bd"] = (lin1, B)
    w["lin1_b"] = (col(kw["lin1_b"]), F)
    w["gate_w1a_bd"] = (bd(kw["gate_lin1_w"][:, :D]), B)
    w["w1bRep"] = (np.tile(kw["gate_lin1_w"][:, D:].T.reshape(-1)[None, :], (P, 1)), B)
    w["attlRep"] = (rep1(kw["gate_att_l"]), B)
    w["gateattrRep2"] = (rep2(kw["gate_att_r"]), B)
    w["gate_w2_bd"] = (bd(kw["gate_lin2_w"]), B)
    w["gate_bias"] = (col(kw["gate_bias"]), F)
    w["gru0_wih"] = (gru_bd(kw["gru0_wih"]), B)
    w["gru0_whh"] = (gru_bd(kw["gru0_whh"]), B)
    w["gru0_bih"] = (gb(kw["gru0_bih"]), F)
    w["gru0_bhh"] = (gb(kw["gru0_bhh"]), F)
    w["gru0_bsum"] = (gb(kw["gru0_bih"] + kw["gru0_bhh"]), F)
    for l in range(4):
        pre = f"at{l}_"
        w[pre + "wT"] = (bd(kw["atom_lin_w"][l]), B)
        w[pre + "srcRep2"] = (rep2(kw["atom_att_src"][l]), B)
        w[pre + "dstRep2"] = (rep2(kw["atom_att_dst"][l]), B)
        w[pre + "bias"] = (col(kw["atom_bias"][l]), F)
        w[pre + "gru_wih"] = (gru_bd(kw["atom_gru_wih"][l]), B)
        w[pre + "gru_whh"] = (gru_bd(kw["atom_gru_whh"][l]), B)
        w[pre + "gru_bih"] = (gb(kw["atom_gru_bih"][l]), F)
        w[pre + "gru_bhh"] = (gb(kw["atom_gru_bhh"][l]), F)
        w[pre + "gru_bsum"] = (gb(kw["atom_gru_bih"][l] + kw["atom_gru_bhh"][l]), F)
    w["mol_bd"] = (bd(kw["mol_lin_w"]), B)
    w["mol_wT32"] = (kw["mol_lin_w"].T.copy(), F)
    w["molsrcRep2"] = (rep2(kw["mol_att_src"]), B)
    w["moldstCol"] = (kw["mol_att_dst"][:, None].copy(), F)
    w["mol_biasRep"] = (rep1(kw["mol_bias"]), F)
    w["mol_gru_wih"] = (kw["mol_gru_wih"].T.copy(), F)
    w["mol_gru_whh"] = (kw["mol_gru_whh"].T.copy(), F)
    w["mol_gru_bih"] = (gb(kw["mol_gru_bih"])[:D], F)
    w["mol_gru_bhh"] = (gb(kw["mol_gru_bhh"])[:D], F)
    w["mol_gru_bsum"] = (gb(kw["mol_gru_bih"] + kw["mol_gru_bhh"])[:D], F)
    w["lin2_wT"] = (kw["lin2_w"].T.copy(), F)
    w["lin2_b"] = (kw["lin2_b"][:, None].copy(), F)
    out = {}
    for k, (v, tag) in w.items():
        v = np.ascontiguousarray(v, np.float32)
        out[k] = v.astype(BF16) if tag == B else v
    return out


def _build(n_pad, t_tiles, tc_max, CH, wmeta):
    import concourse.bacc as bacc
    import concourse.mybir as mybir
    import concourse.tile as tile
    from concourse.bass import IndirectOffsetOnAxis
    from concourse.masks import make_identity

    dt = mybir.dt
    AF = mybir.ActivationFunctionType
    OP = mybir.AluOpType
    AX = mybir.AxisListType
    BF = dt.bfloat16
    F32 = dt.float32

    HC = n_pad // 2
    HT = t_tiles // 2
    NCH = (HC + 511) // 512
    MXP = max(tc_max[2 * tp] + tc_max[2 * tp + 1] for tp in range(HT))
    # aslot gather split points (pair boundaries), 4 parts
    pair_ch0 = []
    s = 0
    for tp in range(HT):
        pair_ch0.append(s)
        s += tc_max[2 * tp] + tc_max[2 * tp + 1]
    pair_ch0.append(CH)
    asplit_tp = [0, HT // 4, HT // 2, 3 * HT // 4, HT]

    nc = bacc.Bacc("TRN2", target_bir_lowering=False, debug=False, num_devices=NC)

    xfm_h = nc.dram_tensor("xfm", [P, HC], BF, kind="ExternalInput")
    idx_h = nc.dram_tensor("idx", [P, CH], dt.int32, kind="ExternalInput")
    idxd_h = nc.dram_tensor("idxd", [P, CH], dt.int32, kind="ExternalInput")
    idxg_h = nc.dram_tensor("idxg", [P, t_tiles], dt.int32, kind="ExternalInput")
    rel_h = nc.dram_tensor("rel", [P, CH], BF, kind="ExternalInput")
    attr_h = nc.dram_tensor("attr", [P, CH * E_DIM], BF, kind="ExternalInput")
    grel0_h = nc.dram_tensor("grel0", [P, t_tiles], BF, kind="ExternalInput")
    grel1_h = nc.dram_tensor("grel1", [P, t_tiles], BF, kind="ExternalInput")
    iota_h = nc.dram_tensor("iotaRow", [P, P], BF, kind="ExternalInput")
    cst_h = {k: nc.dram_tensor("w_" + k, list(s_), BF if isbf else F32,
                               kind="ExternalInput")
             for k, (s_, isbf) in wmeta.items()}
    y_out = nc.dram_tensor("y", [1, G_LOC], F32, kind="ExternalOutput")

    with tile.TileContext(nc) as tc:
      with (
        tc.tile_pool(name="cst", bufs=1) as cst,
        tc.tile_pool(name="st", bufs=1) as st,
        tc.tile_pool(name="ep", bufs=2) as ep,
        tc.tile_pool(name="sp", bufs=2) as sp,
        tc.tile_pool(name="ps", bufs=2, space="PSUM") as ps,
        tc.tile_pool(name="ps_seg", bufs=2, space="PSUM") as ps_seg,
        tc.tile_pool(name="ps_big", bufs=2, space="PSUM") as ps_big,
        tc.tile_pool(name="dram", bufs=1, space="DRAM") as dp,
      ):
        def load(name):
            h = cst_h[name]
            t = cst.tile(list(h.shape), h.dtype, name="c_" + name)
            nc.sync.dma_start(out=t[:], in_=h[:])
            return t
        Wt = {k: load(k) for k in cst_h}
        idx_sb = cst.tile([P, CH], dt.int32, name="idx_sb")
        nc.sync.dma_start(out=idx_sb[:], in_=idx_h[:])
        idxd_sb = cst.tile([P, CH], dt.int32, name="idxd_sb")
        nc.sync.dma_start(out=idxd_sb[:], in_=idxd_h[:])
        idxg_sb = cst.tile([P, t_tiles], dt.int32, name="idxg_sb")
        nc.sync.dma_start(out=idxg_sb[:], in_=idxg_h[:])
        rel_sb = cst.tile([P, CH], BF, name="rel_sb")
        nc.sync.dma_start(out=rel_sb[:], in_=rel_h[:])
        attr_sb = cst.tile([P, CH * E_DIM], BF, name="attr_sb")
        nc.sync.dma_start(out=attr_sb[:], in_=attr_h[:])
        grel_sb = [cst.tile([P, t_tiles], BF, name=f"grel{k}_sb") for k in range(2)]
        nc.sync.dma_start(out=grel_sb[0][:], in_=grel0_h[:])
        nc.sync.dma_start(out=grel_sb[1][:], in_=grel1_h[:])
        iota_sb = cst.tile([P, P], BF, name="iota_sb")
        nc.sync.dma_start(out=iota_sb[:], in_=iota_h[:])
        identb = cst.tile([P, P], BF, name="identb")
        make_identity(nc, identb[:])
        ident = cst.tile([P, P], F32, name="ident")
        make_identity(nc, ident[:])

        XC = st.tile([P, HC], F32, name="XC")     # node state fp32 (fm)
        XCb = st.tile([P, HC], BF, name="XCb")    # bf16 copy for matmuls
        HXb = st.tile([P, HC], BF, name="HXb")    # xt / z / agg / h (bf16, fm)
        ad_nm = st.tile([P, t_tiles], F32, name="ad_nm")
        row_all = st.tile([P, HT * 2 * W], BF, name="row_all")
        tbl_locs = [dp.tile([n_pad, W], BF, name=f"tbl_loc{i}") for i in range(5)]
        tbl_alls = [dp.tile([NC * n_pad, W], BF, addr_space="Shared",
                            name=f"tbl_all{i}") for i in range(5)]
        ads_locs = [dp.tile([n_pad, 1], BF, name=f"ads_loc{i}") for i in range(5)]
        ads_alls = [dp.tile([NC * n_pad, 1], BF, addr_space="Shared",
                            name=f"ads_all{i}") for i in range(5)]
        agr_drams = [dp.tile([2 * P, 1], F32, name=f"agr_dram{i}") for i in range(3)]

        def mm_node(dst, wkey, srcb, act=AF.Copy, bias=None, alpha=0.0):
            for ci in range(NCH):
                c0, c1 = ci * 512, min((ci + 1) * 512, HC)
                pt = ps_big.tile([P, 512], F32, name="mmp", tag="mmp")
                nc.tensor.matmul(pt[:, 0:c1 - c0], Wt[wkey][:], srcb[:, c0:c1],
                                 start=True, stop=True)
                b = Wt[bias][:, 0:1] if bias else 0.0
                nc.scalar.activation(dst[:, c0:c1], pt[:, 0:c1 - c0],
                                     act, bias=b, alpha=alpha)

        def elu_inplace(t_fm, bias):
            for ci in range(NCH):
                c0, c1 = ci * 512, min((ci + 1) * 512, HC)
                w_ = c1 - c0
                v = t_fm[:, c0:c1]
                tin = sp.tile([P, 512], F32, name="eluin", tag="eluin")
                nc.vector.tensor_scalar(out=tin[:, 0:w_], in0=v,
                                        scalar1=Wt[bias][:, 0:1],
                                        scalar2=None, op0=OP.add)
                r = sp.tile([P, 512], F32, name="elur", tag="elur")
                nc.scalar.activation(r[:, 0:w_], tin[:, 0:w_], AF.Relu)
                nc.vector.tensor_sub(tin[:, 0:w_], tin[:, 0:w_], r[:, 0:w_])
                nc.scalar.activation(tin[:, 0:w_], tin[:, 0:w_], AF.Exp)
                nc.vector.scalar_tensor_tensor(
                    out=v, in0=tin[:, 0:w_], scalar=-1.0,
                    in1=r[:, 0:w_], op0=OP.add, op1=OP.add)

        def gru_relu(pre):
            wih = Wt[pre + "wih"]
            whh = Wt[pre + "whh"]
            bs = Wt[pre + "bsum"]
            for ci in range(NCH):
                c0, c1 = ci * 512, min((ci + 1) * 512, HC)
                w_ = c1 - c0
                def gate2(g):
                    pt = ps_big.tile([P, 512], F32, name="grup", tag="mmp")
                    nc.tensor.matmul(pt[:, 0:w_], wih[:, g * P:(g + 1) * P],
                                     HXb[:, c0:c1], start=True, stop=False)
                    nc.tensor.matmul(pt[:, 0:w_], whh[:, g * P:(g + 1) * P],
                                     XCb[:, c0:c1], start=False, stop=True)
                    return pt
                pr = gate2(0)
                r = sp.tile([P, 512], F32, name="grur", tag="grur")
                nc.scalar.activation(r[:, 0:w_], pr[:, 0:w_], AF.Sigmoid,
                                     bias=bs[:, 0:1])
                pz = gate2(1)
                z = sp.tile([P, 512], F32, name="gruz", tag="gruz")
                nc.scalar.activation(z[:, 0:w_], pz[:, 0:w_], AF.Sigmoid,
                                     bias=bs[:, 1:2])
                pin = ps_big.tile([P, 512], F32, name="grupi", tag="mmp")
                nc.tensor.matmul(pin[:, 0:w_], wih[:, 2 * P:3 * P],
                                 HXb[:, c0:c1], start=True, stop=True)
                phn = ps_big.tile([P, 512], F32, name="gruph", tag="mmp")
                nc.tensor.matmul(phn[:, 0:w_], whh[:, 2 * P:3 * P],
                                 XCb[:, c0:c1], start=True, stop=True)
                hn = sp.tile([P, 512], F32, name="gruhn", tag="gruhn", bufs=1)
                nc.vector.tensor_scalar(out=hn[:, 0:w_], in0=phn[:, 0:w_],
                                        scalar1=Wt[pre + "bhh"][:, 2:3],
                                        scalar2=None, op0=OP.add)
                nc.vector.tensor_mul(hn[:, 0:w_], hn[:, 0:w_], r[:, 0:w_])
                nc.vector.tensor_tensor(out=hn[:, 0:w_], in0=hn[:, 0:w_],
                                        in1=pin[:, 0:w_], op=OP.add)
                n_t = sp.tile([P, 512], F32, name="grun", tag="grun", bufs=1)
                nc.scalar.activation(n_t[:, 0:w_], hn[:, 0:w_], AF.Tanh,
                                     bias=Wt[pre + "bih"][:, 2:3])
                d_t = sp.tile([P, 512], F32, name="grud", tag="grud", bufs=1)
                nc.vector.tensor_sub(d_t[:, 0:w_], XC[:, c0:c1], n_t[:, 0:w_])
                nc.vector.tensor_mul(d_t[:, 0:w_], d_t[:, 0:w_], z[:, 0:w_])
                nc.vector.tensor_tensor(out=d_t[:, 0:w_], in0=d_t[:, 0:w_],
                                        in1=n_t[:, 0:w_], op=OP.add)
                nc.scalar.activation(XC[:, c0:c1], d_t[:, 0:w_], AF.Relu)

        def build_table(srcb, srcRep2, dstRep2, ad_from_xc, li):
            for tp in range(HT):
                cc = tp * P
                pT = ps.tile([P, P], BF, name="tabT", tag="miscP")
                nc.tensor.transpose(out=pT[:], in_=srcb[:, cc:cc + P],
                                    identity=identb[:])
                row3 = row_all[:, tp * 2 * W:(tp + 1) * 2 * W].rearrange(
                    "p (h w) -> p h w", w=W)
                nc.vector.tensor_copy(
                    out=row3[:, :, 0:D],
                    in_=pT[:].rearrange("q (h f) -> q h f", f=D))
                if srcRep2 is None:
                    nc.vector.memset(row3[:, :, D:W], 0.0)
                else:
                    tmp = sp.tile([P, P], F32, name="tabm", tag="tabm")
                    nc.vector.tensor_tensor(out=tmp[:], in0=pT[:],
                                            in1=Wt[srcRep2][:], op=OP.mult)
                    asr = sp.tile([P, 2], F32, name="asr", tag="asr")
                    nc.vector.tensor_reduce(asr[:].unsqueeze(2),
                                            tmp[:].rearrange("q (h f) -> q h f", f=D),
                                            axis=AX.X, op=OP.add)
                    nc.vector.tensor_copy(out=row3[:, :, D:W],
                                          in_=asr[:].unsqueeze(2))
                if ad_from_xc:
                    pTx = ps.tile([P, P], BF, name="tabTx", tag="miscP")
                    nc.tensor.transpose(out=pTx[:], in_=XCb[:, cc:cc + P],
                                        identity=identb[:])
                    dsrc = pTx
                else:
                    dsrc = pT
                tmp2 = sp.tile([P, P], F32, name="tabm2", tag="tabm2")
                nc.vector.tensor_tensor(out=tmp2[:], in0=dsrc[:],
                                        in1=Wt[dstRep2][:], op=OP.mult)
                nc.vector.tensor_reduce(ad_nm[:, 2 * tp:2 * tp + 2].unsqueeze(2),
                                        tmp2[:].rearrange("q (h f) -> q h f", f=D),
                                        axis=AX.X, op=OP.add)
            nc.sync.dma_start(
                out=tbl_locs[li][:].rearrange("(tp h p) w -> p tp h w", h=2, p=P),
                in_=row_all[:])
            adc = sp.tile([P, t_tiles], BF, name="adc", tag="adc")
            nc.vector.tensor_copy(out=adc[:], in_=ad_nm[:])
            nc.sync.dma_start(
                out=ads_locs[li][:].rearrange("(j p) one -> p j one", p=P),
                in_=adc[:])
            nc.gpsimd.collective_compute(
                "AllGather", mybir.AluOpType.bypass,
                replica_groups=[list(range(NC))],
                ins=[ads_locs[li].opt()], outs=[ads_alls[li].opt()])
            nc.gpsimd.collective_compute(
                "AllGather", mybir.AluOpType.bypass,
                replica_groups=[list(range(NC))],
                ins=[tbl_locs[li].opt()], outs=[tbl_alls[li].opt()])

        def edge_phase(is_gate, li):
            """Message round; writes agg (normalized, bf16) into HXb (fm)."""
            aslot = ep.tile([P, CH], BF, name="aslot", tag="aslot", bufs=1)
            def aslot_part(i):
                c0, c1 = pair_ch0[asplit_tp[i]], pair_ch0[asplit_tp[i + 1]]
                if c1 > c0:
                    nc.gpsimd.indirect_dma_start(
                        out=aslot[:, c0:c1], out_offset=None, in_=ads_alls[li][:],
                        in_offset=IndirectOffsetOnAxis(
                            ap=idxd_sb[:, c0:c1], axis=0))
            aslot_part(0)
            next_part = 1
            ch0 = 0
            for tp in range(HT):
                if next_part < 4 and tp >= asplit_tp[next_part] - HT // 8:
                    aslot_part(next_part)
                    next_part += 1
                ncha, nchb = tc_max[2 * tp], tc_max[2 * tp + 1]
                chs = ncha + nchb
                gt = ep.tile([P, MXP * W], BF, name="gt", tag="gt", bufs=3)
                nc.gpsimd.indirect_dma_start(
                    out=gt[:, 0:chs * W], out_offset=None, in_=tbl_alls[li][:],
                    in_offset=IndirectOffsetOnAxis(
                        ap=idx_sb[:, ch0:ch0 + chs], axis=0))
                gt3 = gt[:].rearrange("p (c w) -> p c w", w=W)
                s_all = ep.tile([P, MXP * P], BF, name="s_all", tag="s_all")
                nc.vector.tensor_tensor(
                    out=s_all[:, :chs * P],
                    in0=rel_sb[:, ch0:ch0 + chs].unsqueeze(2).to_broadcast(
                        [P, chs, P]),
                    in1=iota_sb[:].unsqueeze(1).to_broadcast([P, chs, P]),
                    op=OP.is_equal)
                q = sp.tile([P, MXP], F32, name="q", tag="q")
                if is_gate:
                    m_all = ep.tile([P, MXP * D], BF, name="m_all", tag="m_all")
                    m3 = m_all[:].rearrange("p (c w) -> p c w", w=D)
                    nc.vector.tensor_copy(out=m3[:, 0:chs, :], in_=gt3[:, 0:chs, 0:D])
                    at3 = attr_sb[:].rearrange("p (c e) -> p c e", e=E_DIM)
                    for j in range(E_DIM):
                        tj = ep.tile([P, MXP * D], BF, name="tj", tag="tj",
                                     bufs=1)
                        nc.vector.tensor_tensor(
                            out=tj[:, :chs * D],
                            in0=at3[:, ch0:ch0 + chs, j:j + 1].to_broadcast(
                                [P, chs, D]),
                            in1=Wt["w1bRep"][:, j * D:(j + 1) * D].unsqueeze(1)
                                .to_broadcast([P, chs, D]),
                            op=OP.mult)
                        nc.vector.tensor_tensor(out=m_all[:, :chs * D],
                                                in0=m_all[:, :chs * D],
                                                in1=tj[:, :chs * D], op=OP.add)
                    nc.scalar.activation(m_all[:, :chs * D], m_all[:, :chs * D],
                                         AF.Prelu, alpha=NEG)
                    lt = ep.tile([P, MXP * D], BF, name="lt", tag="lt", bufs=1)
                    nc.vector.tensor_tensor(
                        out=lt[:, :chs * D], in0=m3[:, 0:chs, :],
                        in1=Wt["attlRep"][:].unsqueeze(1).to_broadcast([P, chs, D]),
                        op=OP.mult)
                    nc.vector.tensor_reduce(q[:, 0:chs].unsqueeze(2),
                                            lt[:, :chs * D].rearrange(
                                                "p (c w) -> p c w", w=D),
                                            axis=AX.X, op=OP.add)
                    nc.vector.tensor_tensor(out=q[:, 0:chs], in0=q[:, 0:chs],
                                            in1=aslot[:, ch0:ch0 + chs], op=OP.add)
                    msg3 = m3
                else:
                    nc.vector.tensor_tensor(out=q[:, 0:chs],
                                            in0=gt3[:, 0:chs, D:W].squeeze(2),
                                            in1=aslot[:, ch0:ch0 + chs], op=OP.add)
                    msg3 = gt3
                e_t = sp.tile([P, MXP], F32, name="e_t", tag="e_t")
                nc.scalar.activation(e_t[:, 0:chs], q[:, 0:chs], AF.Prelu, alpha=NEG)
                nc.scalar.activation(e_t[:, 0:chs], e_t[:, 0:chs], AF.Exp)
                rhs = ep.tile([P, MXP * W], BF, name="rhs", tag="rhs")
                r3 = rhs[:].rearrange("p (c w) -> p c w", w=W)
                nc.vector.tensor_tensor(
                    out=r3[:, 0:chs, 0:D], in0=msg3[:, 0:chs, 0:D],
                    in1=e_t[:, 0:chs].unsqueeze(2).to_broadcast([P, chs, D]),
                    op=OP.mult)
                nc.vector.tensor_copy(out=r3[:, 0:chs, D:W],
                                      in_=e_t[:, 0:chs].unsqueeze(2))
                aggb = sp.tile([P, P], BF, name="aggb", tag="aggb")
                for h, nch in ((0, ncha), (1, nchb)):
                    c0c = 0 if h == 0 else ncha
                    pseg = ps_seg.tile([P, W], F32, name="pseg", tag="pseg")
                    for i in range(nch):
                        nc.tensor.matmul(pseg[:],
                                         s_all[:, (c0c + i) * P:(c0c + i + 1) * P],
                                         rhs[:, (c0c + i) * W:(c0c + i + 1) * W],
                                         start=(i == 0), stop=(i == nch - 1))
                    sn = sp.tile([P, 1], F32, name="sn", tag="sn")
                    nc.vector.tensor_single_scalar(out=sn[:], in_=pseg[:, D:W],
                                                   scalar=1e-16, op=OP.add)
                    rcp = sp.tile([P, 1], F32, name="rcp", tag="rcp")
                    nc.vector.reciprocal(rcp[:], sn[:])
                    nc.vector.tensor_tensor(out=aggb[:, h * D:(h + 1) * D],
                                            in0=pseg[:, 0:D],
                                            in1=rcp[:].to_broadcast([P, D]),
                                            op=OP.mult)
                pT2 = ps.tile([P, P], F32, name="aggT", tag="miscP")
                nc.tensor.matmul(pT2[:], aggb[:], identb[:], start=True, stop=True)
                nc.vector.tensor_copy(out=HXb[:, tp * P:(tp + 1) * P], in_=pT2[:])
                ch0 += chs

        # ================= forward =================
        for ci in range(NCH):
            c0, c1 = ci * 512, min((ci + 1) * 512, HC)
            xin = sp.tile([P, 512], BF, name="xin", tag="xin")
            nc.sync.dma_start(out=xin[:, :c1 - c0], in_=xfm_h[:, c0:c1])
            pt = ps_big.tile([P, 512], F32, name="mmp0", tag="mmp")
            nc.tensor.matmul(pt[:, 0:c1 - c0], Wt["lin1_bd"][:],
                             xin[:, 0:c1 - c0], start=True, stop=True)
            nc.scalar.activation(XC[:, c0:c1], pt[:, 0:c1 - c0],
                                 AF.Prelu, bias=Wt["lin1_b"][:, 0:1], alpha=NEG)
        nc.vector.tensor_copy(out=XCb[:], in_=XC[:])
        # --- GATEConv ---
        mm_node(HXb, "gate_w1a_bd", XCb)                  # z (bf16)
        build_table(HXb, None, "gateattrRep2", True, 0)
        edge_phase(True, 0)
        mm_node(HXb, "gate_w2_bd", HXb)                   # W2 @ agg, in place
        elu_inplace(HXb, "gate_bias")
        gru_relu("gru0_")
        nc.vector.tensor_copy(out=XCb[:], in_=XC[:])
        # --- atom layers ---
        for l in range(4):
            pre = f"at{l}_"
            mm_node(HXb, pre + "wT", XCb)                 # xt (bf16)
            build_table(HXb, pre + "srcRep2", pre + "dstRep2", False, 1 + l)
            edge_phase(False, 1 + l)
            elu_inplace(HXb, pre + "bias")
            gru_relu(pre + "gru_")
            nc.vector.tensor_copy(out=XCb[:], in_=XC[:])

        # ================= readout =================
        # xs = mol_w @ xc (bf16, into HXb); then node-major xs (into row_all)
        mm_node(HXb, "mol_bd", XCb)
        asrc_nm = st.tile([P, t_tiles], F32, name="asrc_nm")
        for tp in range(HT):
            pT = ps.tile([P, P], BF, name="xsT", tag="miscP")
            nc.tensor.transpose(out=pT[:], in_=HXb[:, tp * P:(tp + 1) * P],
                                identity=identb[:])
            nc.vector.tensor_copy(out=row_all[:, tp * 2 * D:(tp + 1) * 2 * D],
                                  in_=pT[:])
            tmp = sp.tile([P, P], F32, name="xsm", tag="tabm")
            nc.vector.tensor_tensor(out=tmp[:], in0=pT[:],
                                    in1=Wt["molsrcRep2"][:], op=OP.mult)
            nc.vector.tensor_reduce(asrc_nm[:, 2 * tp:2 * tp + 2].unsqueeze(2),
                                    tmp[:].rearrange("q (h f) -> q h f", f=D),
                                    axis=AX.X, op=OP.add)
        # node-major xc: transpose XCb in place (fm no longer needed)
        for tp in range(HT):
            pT = ps.tile([P, P], BF, name="xcT", tag="miscP")
            nc.tensor.transpose(out=pT[:], in_=XCb[:, tp * P:(tp + 1) * P],
                                identity=identb[:])
            nc.vector.tensor_copy(out=XCb[:, tp * P:(tp + 1) * P], in_=pT[:])
        xc_nm = XCb
        OUTT = st.tile([D, 2 * P], F32, name="OUTT")
        for k in range(2):
            pg = ps_seg.tile([P, D], F32, name="pg", tag="pseg")
            for tp in range(HT):
                sgp = sp.tile([P, 2 * P], BF, name="sgp", tag="sgp")
                nc.vector.tensor_tensor(
                    out=sgp[:],
                    in0=grel_sb[k][:, 2 * tp:2 * tp + 2].unsqueeze(2).to_broadcast(
                        [P, 2, P]),
                    in1=iota_sb[:].unsqueeze(1).to_broadcast([P, 2, P]),
                    op=OP.is_equal)
                for h in range(2):
                    j = 2 * tp + h
                    nc.tensor.matmul(pg[:], sgp[:, h * P:(h + 1) * P],
                                     xc_nm[:, j * D:(j + 1) * D],
                                     start=(j == 0), stop=(j == t_tiles - 1))
            og = sp.tile([P, D], F32, name="og", tag="og")
            nc.scalar.activation(og[:], pg[:], AF.Relu)
            pTo = ps.tile([D, P], F32, name="ogT", tag="miscP")
            nc.tensor.transpose(out=pTo[:], in_=og[:], identity=ident[:])
            nc.vector.tensor_copy(out=OUTT[:, k * P:(k + 1) * P], in_=pTo[:])
        HG = st.tile([D, 2 * P], F32, name="HG")
        for ts in range(3):
            pxd = ps_big.tile([D, 512], F32, name="xdp", tag="mmp")
            nc.tensor.matmul(pxd[:, 0:2 * P], Wt["mol_wT32"][:], OUTT[:],
                             start=True, stop=True)
            xds = sp.tile([D, 2 * P], F32, name="xds", tag="xds")
            nc.vector.tensor_copy(out=xds[:], in_=pxd[:, 0:2 * P])
            pag = ps.tile([1, 2 * P], F32, name="agp", tag="rowP")
            nc.tensor.matmul(pag[:], Wt["moldstCol"][:], xds[:],
                             start=True, stop=True)
            agr = sp.tile([1, 2 * P], F32, name="agr", tag="agr")
            nc.vector.tensor_copy(out=agr[:], in_=pag[:])
            nc.sync.dma_start(out=agr_drams[ts][:], in_=agr[:])
            ag_nm = sp.tile([P, t_tiles], F32, name="ag_nm", tag="ag_nm")
            nc.gpsimd.indirect_dma_start(
                out=ag_nm[:], out_offset=None, in_=agr_drams[ts][:],
                in_offset=IndirectOffsetOnAxis(ap=idxg_sb[:], axis=0))
            q_all = sp.tile([P, t_tiles], F32, name="q_all", tag="q_all")
            nc.vector.tensor_tensor(out=q_all[:], in0=asrc_nm[:], in1=ag_nm[:],
                                    op=OP.add)
            nc.scalar.activation(q_all[:], q_all[:], AF.Prelu, alpha=NEG)
            nc.scalar.activation(q_all[:], q_all[:], AF.Exp)
            rh = ep.tile([P, t_tiles * W], BF, name="rh", tag="rh", bufs=1)
            rh3 = rh[:].rearrange("p (t w) -> p t w", w=W)
            nc.vector.tensor_tensor(
                out=rh3[:, :, 0:D],
                in0=row_all[:, 0:t_tiles * D].rearrange("p (t f) -> p t f", f=D),
                in1=q_all[:].unsqueeze(2).to_broadcast([P, t_tiles, D]),
                op=OP.mult)
            nc.vector.tensor_copy(out=rh3[:, :, D:W], in_=q_all[:].unsqueeze(2))
            for k in range(2):
                psg = ps_seg.tile([P, W], F32, name="psg", tag="pseg")
                for tp in range(HT):
                    sgp = sp.tile([P, 2 * P], BF, name="sgp2", tag="sgp")
                    nc.vector.tensor_tensor(
                        out=sgp[:],
                        in0=grel_sb[k][:, 2 * tp:2 * tp + 2].unsqueeze(2)
                            .to_broadcast([P, 2, P]),
                        in1=iota_sb[:].unsqueeze(1).to_broadcast([P, 2, P]),
                        op=OP.is_equal)
                    for h in range(2):
                        j = 2 * tp + h
                        nc.tensor.matmul(psg[:], sgp[:, h * P:(h + 1) * P],
                                         rh[:, j * W:(j + 1) * W],
                                         start=(j == 0), stop=(j == t_tiles - 1))
                sn = sp.tile([P, 1], F32, name="sng", tag="sn")
                nc.vector.tensor_single_scalar(out=sn[:], in_=psg[:, D:W],
                                               scalar=1e-16, op=OP.add)
                rcp = sp.tile([P, 1], F32, name="rcpg", tag="rcp")
                nc.vector.reciprocal(rcp[:], sn[:])
                aggg = sp.tile([P, D], F32, name="aggg", tag="aggg")
                nc.vector.tensor_tensor(out=aggg[:], in0=psg[:, 0:D],
                                        in1=rcp[:].to_broadcast([P, D]), op=OP.mult)
                nc.vector.tensor_tensor(out=aggg[:], in0=aggg[:],
                                        in1=Wt["mol_biasRep"][:], op=OP.add)
                r = sp.tile([P, D], F32, name="rg", tag="rg")
                nc.scalar.activation(r[:], aggg[:], AF.Relu)
                xm = sp.tile([P, D], F32, name="xmg", tag="xmg")
                nc.vector.tensor_sub(xm[:], aggg[:], r[:])
                nc.scalar.activation(xm[:], xm[:], AF.Exp)
                nc.vector.scalar_tensor_tensor(out=aggg[:], in0=xm[:], scalar=-1.0,
                                               in1=r[:], op0=OP.add, op1=OP.add)
                pTh = ps.tile([D, P], F32, name="hgT", tag="miscP")
                nc.tensor.transpose(out=pTh[:], in_=aggg[:], identity=ident[:])
                nc.vector.tensor_copy(out=HG[:, k * P:(k + 1) * P], in_=pTh[:])
            wih = Wt["mol_gru_wih"]
            whh = Wt["mol_gru_whh"]
            bs = Wt["mol_gru_bsum"]
            def gate2g(g):
                pt = ps_big.tile([D, 512], F32, name="ggp", tag="mmp")
                nc.tensor.matmul(pt[:, 0:2 * P], wih[:, g * D:(g + 1) * D], HG[:],
                                 start=True, stop=False)
                nc.tensor.matmul(pt[:, 0:2 * P], whh[:, g * D:(g + 1) * D], OUTT[:],
                                 start=False, stop=True)
                return pt
            prg = gate2g(0)
            rg2 = sp.tile([D, 2 * P], F32, name="ggr", tag="ggr")
            nc.scalar.activation(rg2[:], prg[:, 0:2 * P], AF.Sigmoid, bias=bs[:, 0:1])
            pzg = gate2g(1)
            zg = sp.tile([D, 2 * P], F32, name="ggz", tag="ggz")
            nc.scalar.activation(zg[:], pzg[:, 0:2 * P], AF.Sigmoid, bias=bs[:, 1:2])
            pig = ps_big.tile([D, 512], F32, name="ggpi", tag="mmp")
            nc.tensor.matmul(pig[:, 0:2 * P], wih[:, 2 * D:3 * D], HG[:],
                             start=True, stop=True)
            phg = ps_big.tile([D, 512], F32, name="ggph", tag="mmp")
            nc.tensor.matmul(phg[:, 0:2 * P], whh[:, 2 * D:3 * D], OUTT[:],
                             start=True, stop=True)
            hng = sp.tile([D, 2 * P], F32, name="gghn", tag="gghn")
            nc.vector.tensor_scalar(out=hng[:], in0=phg[:, 0:2 * P],
                                    scalar1=Wt["mol_gru_bhh"][:, 2:3],
                                    scalar2=None, op0=OP.add)
            nc.vector.tensor_mul(hng[:], hng[:], rg2[:])
            nc.vector.tensor_tensor(out=hng[:], in0=hng[:], in1=pig[:, 0:2 * P],
                                    op=OP.add)
            ng = sp.tile([D, 2 * P], F32, name="ggn", tag="ggn")
            nc.scalar.activation(ng[:], hng[:], AF.Tanh,
                                 bias=Wt["mol_gru_bih"][:, 2:3])
            dg = sp.tile([D, 2 * P], F32, name="ggd", tag="ggd")
            nc.vector.tensor_sub(dg[:], OUTT[:], ng[:])
            nc.vector.tensor_mul(dg[:], dg[:], zg[:])
            nc.vector.tensor_tensor(out=dg[:], in0=dg[:], in1=ng[:], op=OP.add)
            nc.scalar.activation(OUTT[:], dg[:], AF.Relu)
        py = ps.tile([1, 2 * P], F32, name="py", tag="rowP")
        nc.tensor.matmul(py[:], Wt["lin2_wT"][:], OUTT[:], start=True, stop=True)
        ysb = sp.tile([1, 2 * P], F32, name="ysb", tag="ysb")
        nc.vector.tensor_scalar(out=ysb[:], in0=py[:], scalar1=Wt["lin2_b"][0:1, 0:1],
                                scalar2=None, op0=OP.add)
        nc.sync.dma_start(out=y_out[:], in_=ysb[0:1, 0:G_LOC])
    nc.compile()
    return nc


_CACHE = {}


def kernel(**inputs):
    from concourse.bass_utils import run_bass_kernel_spmd
    x = np.asarray(inputs["x"], np.float32)
    ei = np.asarray(inputs["edge_index"])
    ea = np.asarray(inputs["edge_attr"], np.float32)
    bt = np.asarray(inputs["batch"])
    per, n_pad, t_tiles, tc_max, CH = _prep(x, ei, ea, bt)
    weights = _mk_weights({k: np.asarray(v, np.float32) for k, v in inputs.items()
                           if k not in ("x", "edge_index", "edge_attr", "batch")})
    key = (n_pad, CH, tuple(tc_max))
    if key not in _CACHE:
        _CACHE[key] = _build(n_pad, t_tiles, tc_max, CH,
                             {k: (v.shape, v.dtype == BF16)
                              for k, v in weights.items()})
    nc = _CACHE[key]
    iota = np.tile(np.arange(P).astype(np.float32)[None, :], (P, 1)).astype(BF16)
    in_maps = []
    for c in range(NC):
        m = dict(xfm=per[c]["xfm"], idx=per[c]["idx"], idxd=per[c]["idxd"],
                 idxg=per[c]["idxg"], rel=per[c]["rel"], attr=per[c]["attr"],
                 grel0=per[c]["grel0"], grel1=per[c]["grel1"], iotaRow=iota)
        for k, v in weights.items():
            m["w_" + k] = v
        in_maps.append(m)
    res = run_bass_kernel_spmd(nc, in_maps, core_ids=list(range(NC)))
    return np.concatenate([res.results[c]["y"][0] for c in range(NC)]).astype(np.float32)
